# revision 2
# baseline (speedup 1.0000x reference)
"""LSTM encoder with EOS-freeze for Trainium2, data-parallel over batch on 8 cores.

Strategy
--------
Inputs are one-hot, so x @ Wi is a row-gather of Wi done with indirect DMA on
device (32 rolling SBUF slots, each refilled 32 steps ahead). The recurrent
h @ Wh runs on the tensor engine with Wh as 64 fp16 [128,128] stationary
tiles and h.T chunks as the [128,16] moving operand, producing z transposed:
per-gate PSUM banks [128 partitions = feature % 128, 16*tile + b].

Each step's x-part enters PSUM via PE transposes of the gathered rows that
OPEN the step's accumulation groups (start=True) at the top of the step:
they need no h, so the PE runs them inside the previous step's tail wait,
and the h-matmuls then accumulate on top. Gates are ordered (g, f, i, o)
host-side and the matmuls run tile-outer, so each gate's activation + DVE
ops overlap the later gates' matmuls: tanh(g) and B = sigma_f*c_prev start
early (c_prev is known at step start), the last c-feeder (i) has the short
suffix A -> c, and only sigma_o || tanh(c) -> h remain after the burst.
PSUM z tiles come from a 2-deep pool so the scheduler's per-tag WAR linkage
lands one step back and the sigmoids get single-wait ACTIVATEs.

The hardware loop body covers 64 steps (3 loop boundaries total; each
For_i boundary costs ~9us in all-engine barrier + DMA drains).

The EOS freeze is handled without any per-step masking: sequences are
independent, so the kernel runs the unmasked recurrence and streams per-step
(c, h) snapshots to DRAM; the frozen value for sequence b is the snapshot at
its first-EOS step, selected during unshard.
"""

import numpy as np

try:
    import concourse  # noqa: F401
except ImportError:
    import sys

    sys.path.insert(0, "/opt/trn_rl_repo")

from contextlib import ExitStack

import concourse.bass as bass
import concourse.tile as tile
from concourse import bacc
from concourse import mybir
from concourse.bass import ds
from concourse.bass_utils import run_bass_kernel_spmd

dt = mybir.dt
Alu = mybir.AluOpType
Act = mybir.ActivationFunctionType

EOS_ID = 1
HID = 512
BATCH, SEQ, VOCAB = 128, 256, 1024
GATES = 4 * HID  # 2048
NCORES = 8
BLOC = BATCH // NCORES  # 16 sequences per core
NT = GATES // 128  # 16 feature tiles of z
NK = HID // 128  # 4 contraction chunks
BODY = 64  # steps per For_i iteration

# Collect profiling info when True (set by test.py; adds trace overhead).
TRACE = False
LAST_RESULTS = None  # BassKernelResults of the last run, for test.py

_PROGRAM = None


def _build_program(seq=SEQ, body=BODY):
    nc = bacc.Bacc("TRN2", debug=False, detect_race_conditions=False)

    wi = nc.declare_dram_parameter("wi", [VOCAB, GATES], dt.float16, isOutput=False)
    ident = nc.declare_dram_parameter("ident", [BLOC, BLOC], dt.float16, isOutput=False)
    wh = nc.declare_dram_parameter("wh", [128, NK * NT * 128], dt.float16, isOutput=False)
    tok = nc.declare_dram_parameter("tok", [BLOC, seq + 2 * body], dt.int32, isOutput=False)
    c_traj = nc.declare_dram_parameter("c_traj", [seq * 128, 64], dt.float32, isOutput=True)
    h_traj = nc.declare_dram_parameter("h_traj", [seq * 128, 64], dt.float16, isOutput=True)

    with tile.TileContext(nc) as tc, ExitStack() as ctx:
        pool = lambda name, bufs, **kw: ctx.enter_context(
            tc.tile_pool(name=name, bufs=bufs, **kw)
        )
        whp = pool("whp", 1)
        tokp = pool("tokp", 1)
        stp = pool("stp", 1)
        hp = pool("hp", 1)
        cp = pool("cp", 1)
        zp_pool = pool("zp", 2, space="PSUM")
        sp = pool("sp", 3)
        gp = pool("gp", 3)
        ap_ = pool("ap", 3)
        bp = pool("bp", 3)
        s2p = pool("s2p", 2)
        tp = pool("tp", 3)

        NSLOT = 32  # ST slots: refilled a half-body (32 steps) ahead

        wh_sb = whp.tile([128, NK * NT * 128], dt.float16, name="wh_sb")
        nc.sync.dma_start(out=wh_sb[:], in_=wh[:, :])
        # tok_cur[:, c] == tokens[iv + NSLOT + c]: the rolling lookahead
        # window read by the in-loop gathers (each prefetches NSLOT steps
        # ahead into slot (s % NSLOT)).
        tok_cur = tokp.tile([BLOC, body], dt.int32, name="tok_cur")
        nc.sync.dma_start(out=tok_cur[:], in_=tok[:, NSLOT : NSLOT + body])
        ptok = tokp.tile([BLOC, NSLOT], dt.int32, name="ptok")
        nc.sync.dma_start(out=ptok[:], in_=tok[:, 0:NSLOT])
        id_sb = tokp.tile([BLOC, BLOC], dt.float16, name="id_sb")
        nc.sync.dma_start(out=id_sb[:], in_=ident[:, :])

        ST = [stp.tile([BLOC, GATES], dt.float16, name=f"st{s}", tag=f"st{s}") for s in range(NSLOT)]
        H = [hp.tile([128, 64], dt.float16, name=f"h{s}", tag=f"h{s}") for s in range(body)]
        C = [cp.tile([128, 64], dt.float32, name=f"c{s}", tag=f"c{s}") for s in range(body)]

        nc.gpsimd.memset(H[body - 1][:], 0.0)
        nc.gpsimd.memset(C[body - 1][:], 0.0)

        def gather_slot(slot, tok_ap):
            # Gather BLOC wi rows (one per sequence) for one timestep into
            # ST[slot][b, :] — row-per-partition, the DGE-supported shape.
            nc.gpsimd.indirect_dma_start(
                out=ST[slot][:],
                out_offset=None,
                in_=wi[:, :],
                in_offset=bass.IndirectOffsetOnAxis(ap=tok_ap, axis=0),
            )

        def gather_xp(s):
            # In-loop: prefetch step s+NSLOT (token col s of the window).
            gather_slot(s % NSLOT, tok_cur[:, s : s + 1])

        for s in range(NSLOT):
            gather_slot(s, ptok[:, s : s + 1])

        # Touch the sigmoid table before the loop so the act-table placement
        # sees it loaded on every path into the loop body (otherwise each
        # iteration pays a ~1.3us ACT_TABLE_LOAD at entry).
        warm = tokp.tile([128, 16], dt.float32, name="warm")
        nc.gpsimd.memset(warm[:], 0.0)
        nc.scalar.activation(out=warm[:], in_=warm[:], func=Act.Sigmoid)
        nc.scalar.activation(out=warm[:], in_=warm[:], func=Act.Tanh)

        def alloc_z():
            # One full PSUM bank per gate (a tile smaller than a bank shares
            # it, and start=True clears whole banks, which would serialize
            # steps). Pool generations (bufs=2) keep the scheduler's per-tag
            # WAR linkage one step back — exactly right now that the x-part
            # matmuls open each step's own accumulation group — and give the
            # sigmoids single-wait ACTIVATEs instead of an EVT+ACT pair on
            # the Scalar queue.
            return [
                zp_pool.tile([128, 512], dt.float32, name=f"z{g}", tag=f"z{g}")
                for g in "gfio"
            ]

        def xp_matmuls(Z, s):
            # x@Wi enters PSUM via PE transpose of the gathered rows: these
            # matmuls need no h, so they overlap the previous step's tail.
            for t in range(NT):
                # start=True on each tile's first matmul clears that bank
                # region's has_written bits; later matmuls accumulate.
                nc.tensor.matmul(
                    out=Z[t // 4][:, 16 * (t % 4) : 16 * (t % 4) + 16],
                    lhsT=ST[s % NSLOT][:, 128 * t : 128 * t + 128],
                    rhs=id_sb[:],
                    start=(t % 4 == 0),
                    stop=False,
                )

        def step(iv, s, Z):
            hprev = H[(s - 1) % body]
            cprev = C[(s - 1) % body]
            # The x-part matmuls OPEN this step's accumulation groups
            # (start=True) in this step's own parity banks. They need no h,
            # so the PE runs them during the previous step's tail wait; the
            # h-matmuls then accumulate on top.
            xp_matmuls(Z, s)
            # Gate order along z-features is (g, i, f, o), four tiles each.
            # Tile-outer / k-inner matmul order completes each gate's PSUM
            # slice early, so its activation + DVE ops run while later gates'
            # matmuls still stream; only sigma_o || c-add -> tanh -> h remain
            # after the burst. TG uses the tanh table directly (it coexists
            # with sigmoid in the ACT table set), dropping a DVE op from the
            # A-chain.
            S = sp.tile([128, 64], dt.float32, name="S", tag="S")
            Si = gp.tile([128, 64], dt.float16, name="Si", tag="Si")
            TG = gp.tile([128, 64], dt.float16, name="TG", tag="TG")
            A = ap_.tile([128, 64], dt.float16, name="A", tag="A")
            B = bp.tile([128, 64], dt.float32, name="B", tag="B")
            T = tp.tile([128, 64], dt.float16, name="T", tag="T")
            So = s2p.tile([128, 64], dt.float16, name="So", tag="So")
            cs = C[s]
            for t in range(NT):
                for k in range(NK):
                    nc.tensor.matmul(
                        out=Z[t // 4][:, 16 * (t % 4) : 16 * (t % 4) + 16],
                        lhsT=wh_sb[:, (k * NT + t) * 128 : (k * NT + t) * 128 + 128],
                        rhs=hprev[:, 16 * k : 16 * k + 16],
                        start=False,
                        stop=(t % 4 == 3 and k == NK - 1),
                    )
                if t == 3:  # g tiles 0-3 complete
                    nc.scalar.activation(out=TG[:], in_=Z[0][:, 0:64], func=Act.Tanh)
                elif t == 7:  # f tiles 4-7 complete: B = sigma_f * c_prev
                    # computes early (c_prev is known at step start). sigma_f
                    # stays fp32: its error compounds multiplicatively
                    # through c.
                    nc.scalar.activation(out=S[:], in_=Z[1][:, 0:64], func=Act.Sigmoid)
                    nc.vector.tensor_tensor(out=B[:], in0=S[:], in1=cprev[:], op=Alu.mult)
                elif t == 11:  # i tiles 8-11 complete: the last c-feeder has
                    # the short suffix A -> c, and tanh(c) still fits inside
                    # the o-gate matmuls.
                    nc.scalar.activation(out=Si[:], in_=Z[2][:, 0:64], func=Act.Sigmoid)
                    nc.vector.tensor_tensor(out=A[:], in0=Si[:], in1=TG[:], op=Alu.mult)
                    nc.vector.tensor_tensor(out=cs[:], in0=A[:], in1=B[:], op=Alu.add)
                    nc.scalar.activation(out=T[:], in_=cs[:], func=Act.Tanh)
            # o tiles 12-15: only sigma_o -> h remains after the burst
            nc.scalar.activation(out=So[:], in_=Z[3][:, 0:64], func=Act.Sigmoid)
            hs = H[s]
            nc.vector.tensor_tensor(out=hs[:], in0=So[:], in1=T[:], op=Alu.mult)

            nc.sync.dma_start(out=c_traj[ds((iv + s) * 128, 128), :], in_=cs[:])
            nc.sync.dma_start(out=h_traj[ds((iv + s) * 128, 128), :], in_=hs[:])
            # Prefetch this slot's xp for the next block (the token table is
            # padded so the final block reads harmless extra rows).
            gather_xp(s)

        with tc.For_i(0, seq, body, hint_engines=(mybir.EngineType.PE,), staggered_reset=False) as iv:
            # Roll the token lookahead window (tok_cur[c] == tokens[iv+32+c]):
            # cols[32:64] restaged at the top (read by this body's gathers
            # s>=32), cols[0:32] restaged mid-body for the NEXT iteration —
            # each half right after its previous readers finish.
            nc.sync.dma_start(out=tok_cur[:, NSLOT:body], in_=tok[:, ds(iv + body, NSLOT)])
            for s in range(body):
                step(iv, s, alloc_z())
                if s == NSLOT - 1:
                    nc.sync.dma_start(
                        out=tok_cur[:, 0:NSLOT], in_=tok[:, ds(iv + body + NSLOT, NSLOT)]
                    )

    nc.finalize()
    return nc


def _get_program():
    global _PROGRAM
    if _PROGRAM is None:
        _PROGRAM = _build_program()
    return _PROGRAM


def _prep_host(inputs, Wi, Wh, b):
    tokens = np.argmax(inputs, axis=-1).astype(np.int32)  # [B, T]
    eos = inputs[:, :, EOS_ID] > 0.5
    any_eos = eos.any(axis=1)
    t_star = np.where(any_eos, eos.argmax(axis=1), SEQ - 1).astype(np.int64)

    # Gate reorder (g, f, i, o): each gate's four z-feature tiles finish
    # early in the tile-outer matmul order, overlapping its tail ops with
    # the remaining matmuls.
    perm = np.concatenate(
        [np.arange(1024, 1536), np.arange(512, 1024), np.arange(0, 512), np.arange(1536, 2048)]
    )
    Wi_re = (Wi.astype(np.float32) + b.astype(np.float32)[None, :])[:, perm]
    Wh_re = Wh.astype(np.float32)[:, perm]

    Wi_dev = np.ascontiguousarray(Wi_re).astype(np.float16)
    # Partition-major: wh[kr, (k*NT+t)*128 + p] = Wh_re[128k+kr, 128t+p]
    Wh_dev = np.ascontiguousarray(
        Wh_re.reshape(NK, 128, NT, 128).transpose(1, 0, 2, 3).reshape(128, NK * NT * 128)
    ).astype(np.float16)
    return tokens, t_star, Wi_dev, Wh_dev


def kernel(inputs, Wi, Wh, b):
    global LAST_RESULTS
    inputs = np.asarray(inputs)
    Wi = np.asarray(Wi)
    Wh = np.asarray(Wh)
    b = np.asarray(b)

    tokens, t_star, Wi_dev, Wh_dev = _prep_host(inputs, Wi, Wh, b)

    in_maps = []
    for n in range(NCORES):
        tokc = tokens[BLOC * n : BLOC * (n + 1)]
        tok_pad = np.concatenate([tokc, np.zeros((BLOC, 2 * BODY), np.int32)], axis=1)
        in_maps.append(
            {
                "wi": Wi_dev,
                "wh": Wh_dev,
                "tok": np.ascontiguousarray(tok_pad),
                "ident": np.eye(BLOC, dtype=np.float16),
            }
        )

    nc = _get_program()
    res = run_bass_kernel_spmd(nc, in_maps, list(range(NCORES)), trace=TRACE)
    LAST_RESULTS = res

    c_out = np.zeros((BATCH, HID), np.float32)
    h_out = np.zeros((BATCH, HID), np.float32)
    for n in range(NCORES):
        ct = res.results[n]["c_traj"].reshape(SEQ, 128, 64)
        ht = res.results[n]["h_traj"].reshape(SEQ, 128, 64).astype(np.float32)
        for bl in range(BLOC):
            g = BLOC * n + bl
            t = int(t_star[g])
            c_out[g] = ct[t][:, bl::BLOC].T.reshape(HID)
            h_out[g] = ht[t][:, bl::BLOC].T.reshape(HID)
    return (c_out, h_out)

